# revision 9
# baseline (speedup 1.0000x reference)
"""Trainium2 Bass kernel for nn_MultiHeadAttention_48395691492101.

Strategy: pure head-parallel sharding across 8 NeuronCores (2 heads/core).
Because the reference reshapes ctx [B,H,T,DV] -> [B,T,H*DV] WITHOUT
transposing, row-block t' in [h*128,(h+1)*128) of the reshaped tensor comes
entirely from head h.  Core c (heads 2c,2c+1) therefore owns output rows
[c*256,(c+1)*256) of every batch, and the output projection needs no
cross-core reduction at all - each core computes its own row block with the
full Wout.

Per core (all matmuls in float32r - fp32 storage, reduced-precision
multiplier, 4x the fp32 matmul rate):
  QKV:   QT/KT/VT [128(2h*64), 2048] = W_pair.T @ xT  (xT pre-transposed on
         host), contraction over D in 8 chunks of 128.
  V:     PE-transpose VT -> V natural [k,64] chunks, with a ones column
         appended so the PV matmul also produces softmax row sums (M=65).
  Attn:  ST chunk [128k, 512q] = KT_blk.T @ QT (two heads concurrently via
         PE row-tiling); causal handled by skipping dead chunks, a
         triangular -8e9 add on diagonal blocks, and zero-filling dead AT
         columns.  exp on ACT (scale=1/8 folded in, no max subtraction -
         guarded by a cheap host-side bound).  PV: ctxT/sums [65, 512q]
         accumulated over k chunks.
  Norm:  reciprocal of sums (packed [128,32] via a DRAM bounce), broadcast
         across partitions with gpsimd.partition_broadcast, applied during
         the strided "YT" gather copies.
  Out:   out rows = YT_chunks.T @ WoutT (+bout via a K=1 matmul), multiplied
         by the dropout mask during PSUM eviction.
"""

import sys

if "/opt/trn_rl_repo" not in sys.path:
    sys.path.insert(0, "/opt/trn_rl_repo")

import numpy as np

B, T, D = 4, 2048, 1024
H, DK, DV = 16, 64, 64
SCALE = np.float32(1.0 / 8.0)
NCORES = 8
HP = H // NCORES          # heads per core = 2
ROWS = HP * (T * DV) // D  # output rows per head pair per batch = 256
NDC = D // 128            # 8 d-chunks
NTG = 4                   # t-groups of 512 for QKV
NQG = 4                   # q-groups of 512
NKC = T // 128            # 16 k-chunks
MASK_NEG = np.float32(-8.0e9)   # becomes -1e9 after *SCALE inside exp

_cache = {}


def _build(causal: bool, debug: bool = False):
    import concourse.tile as tile
    import concourse.mybir as mybir
    from concourse import bacc

    F32 = mybir.dt.float32
    F32R = mybir.dt.float32r
    Exp = mybir.ActivationFunctionType.Exp

    nc = bacc.Bacc("TRN2", target_bir_lowering=False, debug=False,
                   num_devices=NCORES)

    xT_d = nc.dram_tensor("xT", [D, B * T], F32R, kind="ExternalInput").ap()
    wq_d = nc.dram_tensor("wq", [D, 128], F32R, kind="ExternalInput").ap()
    wk_d = nc.dram_tensor("wk", [D, 128], F32R, kind="ExternalInput").ap()
    wv_d = nc.dram_tensor("wv", [D, 128], F32R, kind="ExternalInput").ap()
    bq_d = nc.dram_tensor("bq", [128, 1], F32, kind="ExternalInput").ap()
    bk_d = nc.dram_tensor("bk", [128, 1], F32, kind="ExternalInput").ap()
    bv_d = nc.dram_tensor("bv", [128, 1], F32, kind="ExternalInput").ap()
    wout_d = nc.dram_tensor("wout", [D, D], F32R, kind="ExternalInput").ap()
    bout_d = nc.dram_tensor("bout", [1, D], F32R, kind="ExternalInput").ap()
    drop_d = nc.dram_tensor("drop", [B, ROWS, D], F32, kind="ExternalInput").ap()
    id_d = nc.dram_tensor("idm", [128, 128], F32R, kind="ExternalInput").ap()
    onesr_d = nc.dram_tensor("onesr", [1, 128], F32R, kind="ExternalInput").ap()
    vcol_d = nc.dram_tensor("vcol", [128, 16], F32R, kind="ExternalInput").ap()
    zer_d = nc.dram_tensor("zer", [128, 384], F32R, kind="ExternalInput").ap()
    if causal:
        dmask_d = nc.dram_tensor("dmask", [128, 128], F32,
                                 kind="ExternalInput").ap()
    else:
        maskT_d = nc.dram_tensor("maskT", [T, T], F32, kind="ExternalInput").ap()
    out_d = nc.dram_tensor("out", [B, ROWS, D], F32, kind="ExternalOutput").ap()
    dbg = {}
    if debug:
        for nm, shp in (("dqt", [128, T]), ("dkt", [128, T]), ("dvn0", [128, NKC * 66]),
                        ("dctxu", [128, T]), ("dsums", [2, T]), ("dbcast", [128, T]),
                        ("dat", [128, 512])):
            dbg[nm] = nc.dram_tensor(nm, shp, F32, kind="ExternalOutput").ap()

    with tile.TileContext(nc) as tc:
        with tc.tile_pool(name="const", bufs=1) as cpool, \
             tc.tile_pool(name="perb", bufs=1) as perb, \
             tc.tile_pool(name="stream", bufs=3) as stream, \
             tc.tile_pool(name="psum", bufs=1, space="PSUM") as pp, \
             tc.tile_pool(name="dram", bufs=2, space="DRAM") as dpool:

            # ---- constants ----
            wq_sb = cpool.tile([128, D], F32R)
            wk_sb = cpool.tile([128, D], F32R)
            wv_sb = cpool.tile([128, D], F32R)
            for dc in range(NDC):
                nc.sync.dma_start(wq_sb[:, dc * 128:(dc + 1) * 128],
                                  wq_d[dc * 128:(dc + 1) * 128, :])
                nc.sync.dma_start(wk_sb[:, dc * 128:(dc + 1) * 128],
                                  wk_d[dc * 128:(dc + 1) * 128, :])
                nc.sync.dma_start(wv_sb[:, dc * 128:(dc + 1) * 128],
                                  wv_d[dc * 128:(dc + 1) * 128, :])
            wout_sb = cpool.tile([128, NDC * D], F32R)
            for cc in range(NDC):
                nc.sync.dma_start(wout_sb[:, cc * D:(cc + 1) * D],
                                  wout_d[cc * 128:(cc + 1) * 128, :])
            bq_sb = cpool.tile([128, 1], F32)
            bk_sb = cpool.tile([128, 1], F32)
            bv_sb = cpool.tile([128, 1], F32)
            nc.sync.dma_start(bq_sb[:], bq_d[:])
            nc.sync.dma_start(bk_sb[:], bk_d[:])
            nc.sync.dma_start(bv_sb[:], bv_d[:])
            bout_sb = cpool.tile([1, D], F32R)
            nc.sync.dma_start(bout_sb[:], bout_d[:])
            id_sb = cpool.tile([128, 128], F32R)
            nc.sync.dma_start(id_sb[:], id_d[:])
            ones_row = cpool.tile([1, 128], F32R)
            nc.sync.dma_start(ones_row[:], onesr_d[:])
            if causal:
                dmask_sb = cpool.tile([128, 128], F32)
                nc.sync.dma_start(dmask_sb[:], dmask_d[:])

            for b in range(B):
                # ---------- phase 1: QKV projections ----------
                qt = perb.tile([128, T], F32R, bufs=2)
                kt = perb.tile([128, T], F32R, bufs=2)
                vt = perb.tile([128, T], F32R, bufs=1)
                for tg in range(NTG):
                    xts = []
                    for dc in range(NDC):
                        xt = stream.tile([128, 512], F32R, tag="xt", bufs=10)
                        nc.sync.dma_start(
                            xt[:],
                            xT_d[dc * 128:(dc + 1) * 128,
                                 b * T + tg * 512: b * T + (tg + 1) * 512])
                        xts.append(xt)
                    for w_sb, bias_sb, dst in ((wq_sb, bq_sb, qt),
                                               (wk_sb, bk_sb, kt),
                                               (wv_sb, bv_sb, vt)):
                        ps = pp.tile([128, 512], F32, tag="qkv", bufs=2)
                        for dc in range(NDC):
                            nc.tensor.matmul(
                                ps[:], w_sb[:, dc * 128:(dc + 1) * 128],
                                xts[dc][:], start=(dc == 0),
                                stop=(dc == NDC - 1))
                        nc.vector.tensor_scalar_add(
                            dst[:, tg * 512:(tg + 1) * 512], ps[:], bias_sb[:])

                # ---------- phase 2: V transpose (VT -> natural + ones col) ----------
                vn0 = perb.tile([128, NKC * 66], F32R, bufs=1)
                vn1 = perb.tile([128, NKC * 66], F32R, bufs=1)
                nc.sync.dma_start(
                    vn0.rearrange("p (c w) -> p c w", w=66)[:, :, 64], vcol_d[:])
                nc.sync.dma_start(
                    vn1.rearrange("p (c w) -> p c w", w=66)[:, :, 64], vcol_d[:])
                for kc in range(NKC):
                    tp = pp.tile([128, 128], F32R, tag="qkv", bufs=2)
                    nc.tensor.transpose(tp[:], vt[:, kc * 128:(kc + 1) * 128],
                                        id_sb[:])
                    nc.vector.tensor_copy(vn0[:, kc * 66:kc * 66 + 64],
                                          tp[:, 0:64])
                    nc.vector.tensor_copy(vn1[:, kc * 66:kc * 66 + 64],
                                          tp[:, 64:128])

                if debug and b == 0:
                    nc.sync.dma_start(dbg["dqt"][:], qt[:].bitcast(F32))
                    nc.sync.dma_start(dbg["dkt"][:], kt[:].bitcast(F32))
                    nc.sync.dma_start(dbg["dvn0"][:], vn0[:].bitcast(F32))
                # ---------- phase 3: attention ----------
                ctxu0 = perb.tile([64, T], F32, bufs=1)
                ctxu1 = perb.tile([64, T], F32, bufs=1)
                sums = perb.tile([33, T], F32, bufs=1)
                for qg in range(NQG):
                    kcmax = 4 * qg + 4 if causal else NKC
                    cs0 = pp.tile([65, 512], F32, tag="cs", bufs=2)
                    cs1 = pp.tile([65, 512], F32, tag="cs", bufs=2)
                    for kc in range(kcmax):
                        o = kc - 4 * qg
                        diag = causal and o >= 0
                        live = o * 128 if diag else 0
                        if not causal:
                            mt = stream.tile([128, 512], F32, tag="mt", bufs=3)
                            nc.sync.dma_start(
                                mt[:],
                                maskT_d[kc * 128:(kc + 1) * 128,
                                        qg * 512:(qg + 1) * 512])
                        ats = []
                        for h, (cs, vn) in enumerate(((cs0, vn0), (cs1, vn1))):
                            st = pp.tile([128, 512], F32, tag="st", bufs=3)
                            nc.tensor.matmul(
                                st[:, live:512],
                                kt[64 * h:64 * h + 64,
                                   kc * 128:(kc + 1) * 128],
                                qt[64 * h:64 * h + 64,
                                   qg * 512 + live:(qg + 1) * 512],
                                start=True, stop=True)
                            if diag:
                                nc.vector.tensor_add(
                                    st[:, live:live + 128],
                                    st[:, live:live + 128], dmask_sb[:])
                            elif not causal:
                                nc.vector.tensor_add(st[:], st[:], mt[:])
                            at = stream.tile([128, 512], F32R, tag="at",
                                             bufs=4)
                            nc.scalar.activation(at[:, live:512],
                                                 st[:, live:512], Exp,
                                                 scale=float(SCALE))
                            if live > 0:
                                nc.sync.dma_start(at[:, 0:live],
                                                  zer_d[:, 0:live])
                            if debug and b == 0 and qg == 0 and kc == 0 and h == 0:
                                nc.sync.dma_start(dbg["dat"][:],
                                                  at[:].bitcast(F32))
                            ats.append(at)
                        for h, (cs, vn) in enumerate(((cs0, vn0), (cs1, vn1))):
                            nc.tensor.matmul(
                                cs[:], vn[:, kc * 66:kc * 66 + 65],
                                ats[h][:], start=(kc == 0),
                                stop=(kc == kcmax - 1))
                    for h, cs in enumerate((cs0, cs1)):
                        nc.vector.tensor_copy(
                            (ctxu0 if h == 0 else ctxu1)
                            [:, qg * 512:(qg + 1) * 512], cs[0:64, :])
                        nc.vector.tensor_copy(
                            sums[32 * h:32 * h + 1, qg * 512:(qg + 1) * 512],
                            cs[64:65, :])

                # ---------- phase 4: reciprocal + broadcast ----------
                scrA = dpool.tile([2 * T], F32)
                scrA2 = scrA.rearrange("(h q) -> h q", h=2)
                nc.sync.dma_start(scrA2[0:1, :], sums[0:1, :])
                nc.sync.dma_start(scrA2[1:2, :], sums[32:33, :])
                s128 = stream.tile([128, 32], F32, tag="s128", bufs=2)
                nc.sync.dma_start(s128[:],
                                  scrA.rearrange("(p j) -> p j", j=32))
                r128 = stream.tile([128, 32], F32, tag="r128", bufs=2)
                nc.vector.reciprocal(r128[:], s128[:])
                scrB = dpool.tile([2 * T], F32)
                nc.sync.dma_start(scrB.rearrange("(p j) -> p j", j=32),
                                  r128[:])
                bcast0 = perb.tile([64, T], F32, bufs=1)
                bcast1 = perb.tile([64, T], F32, bufs=1)
                nc.sync.dma_start(bcast0[0:1, :],
                                  scrB.rearrange("(h q) -> h q", h=2)[0:1, :])
                nc.sync.dma_start(bcast1[0:1, :],
                                  scrB.rearrange("(h q) -> h q", h=2)[1:2, :])
                nc.gpsimd.partition_broadcast(bcast0[:, :], bcast0[0:1, :])
                nc.gpsimd.partition_broadcast(bcast1[:, :], bcast1[0:1, :])
                ctxn0 = perb.tile([64, T], F32, bufs=1)
                ctxn1 = perb.tile([64, T], F32, bufs=1)
                nc.vector.tensor_mul(ctxn0[:], ctxu0[:], bcast0[:])
                nc.vector.tensor_mul(ctxn1[:], ctxu1[:], bcast1[:])

                if debug and b == 0:
                    nc.sync.dma_start(dbg["dctxu"][0:64, :], ctxu0[:])
                    nc.sync.dma_start(dbg["dctxu"][64:128, :], ctxu1[:])
                    nc.sync.dma_start(dbg["dsums"][0:1, :], sums[0:1, :])
                    nc.sync.dma_start(dbg["dsums"][1:2, :], sums[32:33, :])
                    nc.sync.dma_start(dbg["dbcast"][0:64, :], bcast0[:])
                    nc.sync.dma_start(dbg["dbcast"][64:128, :], bcast1[:])
                # ---------- phase 5: output projection ----------
                for h in range(HP):
                    cvu = (ctxn0 if h == 0 else ctxn1)[:, :].rearrange(
                        "p (r s) -> p s r", s=16)
                    yts = []
                    for cc in range(NDC):
                        yt = stream.tile([128, 128], F32R, tag="yt", bufs=9)
                        nc.vector.tensor_copy(yt[0:64, :], cvu[:, 2 * cc, :])
                        nc.vector.tensor_copy(yt[64:128, :],
                                              cvu[:, 2 * cc + 1, :])
                        yts.append(yt)
                    for og in range(2):
                        po = pp.tile([128, 512], F32, tag="qkv", bufs=2)
                        for cc in range(NDC):
                            nc.tensor.matmul(
                                po[:], yts[cc][:],
                                wout_sb[:, cc * D + og * 512:
                                        cc * D + og * 512 + 512],
                                start=(cc == 0), stop=False)
                        nc.tensor.matmul(po[:], ones_row[:],
                                         bout_sb[0:1, og * 512:(og + 1) * 512],
                                         start=False, stop=True)
                        dt = stream.tile([128, 512], F32, tag="dt", bufs=2)
                        nc.sync.dma_start(
                            dt[:], drop_d[b, h * 128:(h + 1) * 128,
                                          og * 512:(og + 1) * 512])
                        ost = stream.tile([128, 512], F32, tag="ost", bufs=2)
                        nc.vector.tensor_mul(ost[:], po[:], dt[:])
                        nc.sync.dma_start(
                            out_d[b, h * 128:(h + 1) * 128,
                                  og * 512:(og + 1) * 512], ost[:])

    nc.compile()
    return nc


def _get_program(causal: bool):
    key = ("causal" if causal else "full")
    if key not in _cache:
        _cache[key] = _build(causal)
    return _cache[key]


def _host_fallback(x, attn_mask, Wq, bq, Wk, bk, Wv, bv, Wout, bout,
                   dropout_mask):
    x64 = x.astype(np.float32)
    Q = np.einsum("btd,hdk->bhtk", x64, Wq) + bq[None, :, None, :]
    K = np.einsum("btd,hdk->bhtk", x64, Wk) + bk[None, :, None, :]
    V = np.einsum("btd,hdv->bhtv", x64, Wv) + bv[None, :, None, :]
    scores = np.einsum("bhqk,bhmk->bhqm", Q, K) * SCALE + attn_mask
    scores = scores - scores.max(-1, keepdims=True)
    e = np.exp(scores)
    attn = e / e.sum(-1, keepdims=True)
    ctx = np.einsum("bhqm,bhmv->bhqv", attn, V).reshape(B, T, H * DV)
    out = ctx @ Wout.T + bout
    return (out * dropout_mask).astype(np.float32)


def kernel(x, attn_mask, Wq, bq, Wk, bk, Wv, bv, Wout, bout, dropout_mask):
    from concourse.bass_utils import run_bass_kernel_spmd

    x = np.ascontiguousarray(x, np.float32)
    m = np.asarray(attn_mask, np.float32).reshape(T, T)

    # causality check on the actual mask tensor
    causal = bool((np.tril(m) == 0).all() and
                  (m[np.triu_indices(T, 1)] <= -1e8).all())

    # safety: cheap bound on max |scaled score| -> exp overflow guard
    xf = x.reshape(B * T, D)
    Qa = xf @ Wq.transpose(1, 0, 2).reshape(D, H * DK)
    Ka = xf @ Wk.transpose(1, 0, 2).reshape(D, H * DK)
    Qa = Qa.reshape(B * T, H, DK) + bq[None]
    Ka = Ka.reshape(B * T, H, DK) + bk[None]
    qn = np.linalg.norm(Qa, axis=2).max(0)     # per-head max row norm
    kn = np.linalg.norm(Ka, axis=2).max(0)
    bound = float(SCALE) * float((qn * kn).max())
    if bound > 50.0:
        return _host_fallback(x, attn_mask, Wq, bq, Wk, bk, Wv, bv, Wout,
                              bout, dropout_mask)

    nc = _get_program(causal)

    xT = np.ascontiguousarray(x.transpose(2, 0, 1).reshape(D, B * T))
    woutT = np.ascontiguousarray(np.asarray(Wout, np.float32).T)
    boutr = np.asarray(bout, np.float32).reshape(1, D)
    idm = np.eye(128, dtype=np.float32)
    dmask = np.where(np.arange(128)[None, :] < np.arange(128)[:, None],
                     MASK_NEG, np.float32(0.0)).astype(np.float32)
    maskT = None if causal else np.ascontiguousarray(m.T * np.float32(8.0))
    drop = np.asarray(dropout_mask, np.float32)

    in_maps = []
    for c in range(NCORES):
        h0, h1 = HP * c, HP * c + 1
        im = {
            "xT": xT,
            "wq": np.ascontiguousarray(
                np.concatenate([Wq[h0], Wq[h1]], axis=1), np.float32),
            "wk": np.ascontiguousarray(
                np.concatenate([Wk[h0], Wk[h1]], axis=1), np.float32),
            "wv": np.ascontiguousarray(
                np.concatenate([Wv[h0], Wv[h1]], axis=1), np.float32),
            "bq": np.concatenate([bq[h0], bq[h1]]).reshape(128, 1)
                    .astype(np.float32),
            "bk": np.concatenate([bk[h0], bk[h1]]).reshape(128, 1)
                    .astype(np.float32),
            "bv": np.concatenate([bv[h0], bv[h1]]).reshape(128, 1)
                    .astype(np.float32),
            "wout": woutT,
            "bout": boutr,
            "drop": np.ascontiguousarray(
                drop[:, c * ROWS:(c + 1) * ROWS, :]),
            "idm": idm,
            "onesr": np.ones((1, 128), np.float32),
            "vcol": np.ones((128, 16), np.float32),
            "zer": np.zeros((128, 384), np.float32),
        }
        if causal:
            im["dmask"] = dmask
        else:
            im["maskT"] = maskT
        in_maps.append(im)

    res = run_bass_kernel_spmd(nc, in_maps, list(range(NCORES)))
    out = np.empty((B, T, D), np.float32)
    for c in range(NCORES):
        out[:, c * ROWS:(c + 1) * ROWS, :] = res.results[c]["out"]
    return out


# revision 23
# speedup vs baseline: 24038.9728x; 24038.9728x over previous
"""Trainium2 Bass kernel for nn_MultiHeadAttention_48395691492101.

Strategy: pure head-parallel sharding across 8 NeuronCores (2 heads/core).
Because the reference reshapes ctx [B,H,T,DV] -> [B,T,H*DV] WITHOUT
transposing, row-block t' in [h*128,(h+1)*128) of the reshaped tensor comes
entirely from head h.  Core c (heads 2c,2c+1) therefore owns output rows
[c*256,(c+1)*256) of every batch, and the output projection needs no
cross-core reduction at all - each core computes its own row block with the
full Wout.

Per core (all matmuls in float32r - fp32 storage, reduced-precision
multiplier, 4x the fp32 matmul rate):
  QKV:   QT/KT/VT [128(2h*64), 2048] = W_pair.T @ xT  (xT pre-transposed on
         host), contraction over D in 8 chunks of 128.
  V:     PE-transpose VT -> V natural [k,64] chunks, with a ones column
         appended so the PV matmul also produces softmax row sums (M=65).
  Attn:  ST chunk [128k, 512q] = KT_blk.T @ QT (two heads concurrently via
         PE row-tiling); causal handled by skipping dead chunks, a
         triangular -8e9 add on diagonal blocks, and zero-filling dead AT
         columns.  exp on ACT (scale=1/8 folded in, no max subtraction -
         guarded by a cheap host-side bound).  PV: ctxT/sums [65, 512q]
         accumulated over k chunks.
  Norm:  reciprocal of sums (packed [128,32] via a DRAM bounce), broadcast
         across partitions with gpsimd.partition_broadcast, applied during
         the strided "YT" gather copies.
  Out:   out rows = YT_chunks.T @ WoutT (+bout via a K=1 matmul), multiplied
         by the dropout mask during PSUM eviction.
"""

import sys

if "/opt/trn_rl_repo" not in sys.path:
    sys.path.insert(0, "/opt/trn_rl_repo")

import numpy as np

B, T, D = 4, 2048, 1024
H, DK, DV = 16, 64, 64
SCALE = np.float32(1.0 / 8.0)
NCORES = 8
HP = H // NCORES          # heads per core = 2
ROWS = HP * (T * DV) // D  # output rows per head pair per batch = 256
NDC = D // 128            # 8 d-chunks
NTG = 4                   # t-groups of 512 for QKV
NQG = 4                   # q-groups of 512
NKC = T // 128            # 16 k-chunks
MASK_NEG = np.float32(-8.0e9)   # becomes -1e9 after *SCALE inside exp

_cache = {}


def _build(causal: bool, debug: bool = False):
    import concourse.tile as tile
    import concourse.mybir as mybir
    from concourse import bacc

    F32 = mybir.dt.float32
    F32R = mybir.dt.float32r
    Exp = mybir.ActivationFunctionType.Exp

    nc = bacc.Bacc("TRN2", target_bir_lowering=False, debug=False,
                   num_devices=NCORES)

    xT_d = nc.dram_tensor("xT", [D, B * T], F32R, kind="ExternalInput").ap()
    wq_d = nc.dram_tensor("wq", [D, 128], F32R, kind="ExternalInput").ap()
    wk_d = nc.dram_tensor("wk", [D, 128], F32R, kind="ExternalInput").ap()
    wv_d = nc.dram_tensor("wv", [D, 128], F32R, kind="ExternalInput").ap()
    bq_d = nc.dram_tensor("bq", [128, 1], F32, kind="ExternalInput").ap()
    bk_d = nc.dram_tensor("bk", [128, 1], F32, kind="ExternalInput").ap()
    bv_d = nc.dram_tensor("bv", [128, 1], F32, kind="ExternalInput").ap()
    wout_d = nc.dram_tensor("wout", [D, D], F32R, kind="ExternalInput").ap()
    bout_d = nc.dram_tensor("bout", [1, D], F32R, kind="ExternalInput").ap()
    drop_d = nc.dram_tensor("drop", [B, ROWS, D], F32, kind="ExternalInput").ap()
    id_d = nc.dram_tensor("idm", [128, 128], F32R, kind="ExternalInput").ap()
    onesr_d = nc.dram_tensor("onesr", [1, 128], F32R, kind="ExternalInput").ap()
    vcol_d = nc.dram_tensor("vcol", [128, 32], F32R, kind="ExternalInput").ap()
    zer_d = nc.dram_tensor("zer", [128, 384], F32R, kind="ExternalInput").ap()
    if causal:
        dmask_d = nc.dram_tensor("dmask", [128, 128], F32,
                                 kind="ExternalInput").ap()
    else:
        maskT_d = nc.dram_tensor("maskT", [T, T], F32, kind="ExternalInput").ap()
    out_d = nc.dram_tensor("out", [B, ROWS, D], F32, kind="ExternalOutput").ap()
    dbg = {}
    if debug:
        for nm, shp in (("dqt", [128, T]), ("dkt", [128, T]),
                        ("dvn0", [128, NKC * 66]), ("dctxu", [128, T]),
                        ("dsums", [2, T]), ("dbcast", [128, T]),
                        ("dat", [128, 512])):
            dbg[nm] = nc.dram_tensor(nm, shp, F32, kind="ExternalOutput").ap()

    with tile.TileContext(nc) as tc:
        with tc.tile_pool(name="const", bufs=1) as cpool, \
             tc.tile_pool(name="perb", bufs=1) as perb, \
             tc.tile_pool(name="stream", bufs=3) as stream, \
             tc.tile_pool(name="psum", bufs=1, space="PSUM") as pp, \
             tc.tile_pool(name="dram", bufs=2, space="DRAM") as dpool:

            # ---- constants (one DMA per tensor; wout on the scalar queue) ----
            wq_sb = cpool.tile([128, D], F32R)
            wk_sb = cpool.tile([128, D], F32R)
            wv_sb = cpool.tile([128, D], F32R)
            for w_sb, w_d in ((wq_sb, wq_d), (wk_sb, wk_d), (wv_sb, wv_d)):
                nc.sync.dma_start(
                    w_sb.rearrange("p (dc m) -> p dc m", m=128),
                    w_d.rearrange("(dc p) m -> p dc m", p=128))
            wout_sb = cpool.tile([128, NDC * D], F32R)
            nc.scalar.dma_start(
                wout_sb.rearrange("p (cc o) -> p cc o", o=D),
                wout_d.rearrange("(cc p) o -> p cc o", p=128))
            bq_sb = cpool.tile([128, 1], F32)
            bk_sb = cpool.tile([128, 1], F32)
            bv_sb = cpool.tile([128, 1], F32)
            nc.gpsimd.dma_start(bq_sb[:], bq_d[:])
            nc.gpsimd.dma_start(bk_sb[:], bk_d[:])
            nc.gpsimd.dma_start(bv_sb[:], bv_d[:])
            bout_sb = cpool.tile([1, D], F32R)
            nc.scalar.dma_start(bout_sb[:], bout_d[:])
            id_sb = cpool.tile([128, 128], F32R)
            nc.gpsimd.dma_start(id_sb[:], id_d[:])
            ones_row = cpool.tile([1, 128], F32R)
            nc.scalar.dma_start(ones_row[:], onesr_d[:])
            vcol_sb = cpool.tile([128, 32], F32R)
            nc.gpsimd.dma_start(vcol_sb[:], vcol_d[:])
            zcol_sb = cpool.tile([128, 384], F32R)
            nc.gpsimd.dma_start(zcol_sb[:], zer_d[:])
            if causal:
                dmask_sb = cpool.tile([128, 128], F32)
                nc.gpsimd.dma_start(dmask_sb[:], dmask_d[:])

            for b in range(B):
                # ---------- phase 1: QKV projections ----------
                qt = perb.tile([128, T], F32R, bufs=2)
                kt = perb.tile([128, T], F32R, bufs=2)
                vt = perb.tile([128, T], F32R, bufs=1)
                for tg in range(NTG):
                    xts = []
                    for half in range(2):
                        xt = stream.tile([128, 4 * 512], F32R, tag="xt",
                                         bufs=4)
                        c0 = b * T + tg * 512
                        nc.sync.dma_start(
                            xt.rearrange("p (dc j) -> p dc j", j=512),
                            xT_d.rearrange("(dc p) q -> p dc q", p=128)
                            [:, 4 * half:4 * half + 4, c0:c0 + 512])
                        xts.append(xt)
                    for w_sb, bias_sb, dst in ((wq_sb, bq_sb, qt),
                                               (wk_sb, bk_sb, kt),
                                               (wv_sb, bv_sb, vt)):
                        ps = pp.tile([128, 512], F32, tag="qkv", bufs=2)
                        for dc in range(NDC):
                            nc.tensor.matmul(
                                ps[:], w_sb[:, dc * 128:(dc + 1) * 128],
                                xts[dc // 4][:, (dc % 4) * 512:
                                             (dc % 4) * 512 + 512],
                                start=(dc == 0), stop=(dc == NDC - 1))
                        nc.vector.tensor_scalar_add(
                            dst[:, tg * 512:(tg + 1) * 512], ps[:], bias_sb[:])

                # ---------- phase 2: V transpose ----------
                vnb = perb.tile([128, NKC * 132], F32R, bufs=2)
                nc.vector.tensor_copy(
                    vnb.rearrange("p (c two w) -> p c two w", two=2, w=66)
                    [:, :, :, 64],
                    vcol_sb.rearrange("p (c two) -> p c two", two=2))
                for kc in range(NKC):
                    tp = pp.tile([128, 128], F32R, tag="qkv", bufs=2)
                    nc.tensor.transpose(tp[:], vt[:, kc * 128:(kc + 1) * 128],
                                        id_sb[:])
                    nc.vector.tensor_copy(
                        vnb.rearrange("p (c two w) -> p c two w", two=2, w=66)
                        [:, kc, :, 0:64],
                        tp[:].rearrange("p (two v) -> p two v", two=2))
                if debug and b == 0:
                    nc.sync.dma_start(dbg["dqt"][:], qt[:].bitcast(F32))
                    nc.sync.dma_start(dbg["dkt"][:], kt[:].bitcast(F32))
                    nc.sync.dma_start(
                        dbg["dvn0"][:],
                        vnb.rearrange("p (c two w) -> p two c w", two=2, w=66)
                        [:, 0, :, :].bitcast(F32))

                # ---------- phase 3: attention ----------
                ex0 = perb.tile([65, T], F32, bufs=1)
                ex1 = perb.tile([65, T], F32, bufs=1)
                for qg in range(NQG):
                    kcmax = 4 * qg + 4 if causal else NKC
                    cs0 = pp.tile([65, 512], F32, tag="cs", bufs=2)
                    cs1 = pp.tile([65, 512], F32, tag="cs", bufs=2)
                    for kc in range(kcmax):
                        o = kc - 4 * qg
                        diag = causal and o >= 0
                        live = o * 128 if diag else 0
                        if not causal:
                            mt = stream.tile([128, 512], F32, tag="mt", bufs=3)
                            nc.sync.dma_start(
                                mt[:],
                                maskT_d[kc * 128:(kc + 1) * 128,
                                        qg * 512:(qg + 1) * 512])
                        ats = []
                        for h in range(2):
                            st = pp.tile([128, 512], F32, tag="st", bufs=3)
                            nc.tensor.matmul(
                                st[:, live:512],
                                kt[64 * h:64 * h + 64,
                                   kc * 128:(kc + 1) * 128],
                                qt[64 * h:64 * h + 64,
                                   qg * 512 + live:(qg + 1) * 512],
                                start=True, stop=True)
                            if diag:
                                nc.vector.tensor_add(
                                    st[:, live:live + 128],
                                    st[:, live:live + 128], dmask_sb[:])
                            elif not causal:
                                nc.vector.tensor_add(st[:], st[:], mt[:])
                            at = stream.tile([128, 512], F32R, tag="at",
                                             bufs=3)
                            nc.scalar.activation(at[:, live:512],
                                                 st[:, live:512], Exp,
                                                 scale=float(SCALE))
                            if live > 0:
                                nc.vector.tensor_copy(at[:, 0:live],
                                                      zcol_sb[:, 0:live])
                            if debug and b == 0 and qg == 0 and kc == 0 and h == 0:
                                nc.sync.dma_start(dbg["dat"][:],
                                                  at[:].bitcast(F32))
                            ats.append(at)
                        for h, cs in enumerate((cs0, cs1)):
                            nc.tensor.matmul(
                                cs[:],
                                vnb[:, kc * 132 + 66 * h:
                                    kc * 132 + 66 * h + 65],
                                ats[h][:], start=(kc == 0),
                                stop=(kc == kcmax - 1))
                    for h, cs in enumerate((cs0, cs1)):
                        nc.vector.tensor_copy(
                            (ex0 if h == 0 else ex1)
                            [:, qg * 512:(qg + 1) * 512], cs[:, :])

                # ---------- phase 4: reciprocal + broadcast (no DRAM bounce) ----------
                scrA = dpool.tile([2 * T], F32)
                scrA2 = scrA.rearrange("(h q) -> h q", h=2)
                nc.scalar.dma_start(scrA2[0:1, :], ex0[64:65, :])
                nc.scalar.dma_start(scrA2[1:2, :], ex1[64:65, :])
                s128 = stream.tile([128, 32], F32, tag="s128", bufs=2)
                nc.scalar.dma_start(s128[:],
                                    scrA.rearrange("(p j) -> p j", j=32))
                r128 = stream.tile([128, 32], F32, tag="r128", bufs=2)
                nc.vector.reciprocal(r128[:], s128[:])
                scrB = dpool.tile([2 * T], F32)
                nc.scalar.dma_start(scrB.rearrange("(p j) -> p j", j=32),
                                    r128[:])
                bcast0 = perb.tile([64, T], F32, bufs=1)
                bcast1 = perb.tile([64, T], F32, bufs=1)
                nc.scalar.dma_start(
                    bcast0[0:1, :], scrB.rearrange("(h q) -> h q", h=2)[0:1, :])
                nc.scalar.dma_start(
                    bcast1[0:1, :], scrB.rearrange("(h q) -> h q", h=2)[1:2, :])
                nc.gpsimd.partition_broadcast(bcast0[:, :], bcast0[0:1, :])
                nc.gpsimd.partition_broadcast(bcast1[:, :], bcast1[0:1, :])
                nc.vector.tensor_mul(ex0[0:64, :], ex0[0:64, :], bcast0[:])
                nc.vector.tensor_mul(ex1[0:64, :], ex1[0:64, :], bcast1[:])
                if debug and b == 0:
                    nc.sync.dma_start(dbg["dctxu"][0:64, :], ex0[0:64, :])
                    nc.sync.dma_start(dbg["dctxu"][64:128, :], ex1[0:64, :])
                    nc.sync.dma_start(dbg["dsums"][0:1, :], ex0[64:65, :])
                    nc.sync.dma_start(dbg["dsums"][1:2, :], ex1[64:65, :])
                    nc.sync.dma_start(dbg["dbcast"][0:64, :], bcast0[:])
                    nc.sync.dma_start(dbg["dbcast"][64:128, :], bcast1[:])

                # ---------- phase 5: output projection ----------
                for h in range(HP):
                    cvu = (ex0 if h == 0 else ex1)[0:64, :].rearrange(
                        "p (r s2 two) -> p two s2 r", two=2, s2=8)
                    yts = stream.tile([128, NDC * 128], F32R, tag="yt",
                                      bufs=1)
                    ytv = yts.rearrange("p (c r) -> p c r", r=128)
                    nc.vector.tensor_copy(ytv[0:64, :, :], cvu[:, 0, :, :])
                    nc.vector.tensor_copy(ytv[64:128, :, :], cvu[:, 1, :, :])
                    dt2 = stream.tile([128, D], F32, tag="dt", bufs=1)
                    nc.scalar.dma_start(
                        dt2[:], drop_d[b, h * 128:(h + 1) * 128, :])
                    ost2 = stream.tile([128, D], F32, tag="ost", bufs=2)
                    for og in range(2):
                        po = pp.tile([128, 512], F32, tag="po", bufs=1)
                        for cc in range(NDC):
                            nc.tensor.matmul(
                                po[:], yts[:, cc * 128:(cc + 1) * 128],
                                wout_sb[:, cc * D + og * 512:
                                        cc * D + og * 512 + 512],
                                start=(cc == 0), stop=False)
                        nc.tensor.matmul(po[:], ones_row[:],
                                         bout_sb[0:1, og * 512:(og + 1) * 512],
                                         start=False, stop=True)
                        nc.vector.tensor_mul(
                            ost2[:, og * 512:(og + 1) * 512], po[:],
                            dt2[:, og * 512:(og + 1) * 512])
                    nc.scalar.dma_start(
                        out_d[b, h * 128:(h + 1) * 128, :], ost2[:])

    nc.compile()
    return nc


def _get_program(causal: bool):
    key = ("causal" if causal else "full")
    if key not in _cache:
        _cache[key] = _build(causal)
    return _cache[key]


def _host_fallback(x, attn_mask, Wq, bq, Wk, bk, Wv, bv, Wout, bout,
                   dropout_mask):
    x64 = x.astype(np.float32)
    Q = np.einsum("btd,hdk->bhtk", x64, Wq) + bq[None, :, None, :]
    K = np.einsum("btd,hdk->bhtk", x64, Wk) + bk[None, :, None, :]
    V = np.einsum("btd,hdv->bhtv", x64, Wv) + bv[None, :, None, :]
    scores = np.einsum("bhqk,bhmk->bhqm", Q, K) * SCALE + attn_mask
    scores = scores - scores.max(-1, keepdims=True)
    e = np.exp(scores)
    attn = e / e.sum(-1, keepdims=True)
    ctx = np.einsum("bhqm,bhmv->bhqv", attn, V).reshape(B, T, H * DV)
    out = ctx @ Wout.T + bout
    return (out * dropout_mask).astype(np.float32)


def kernel(x, attn_mask, Wq, bq, Wk, bk, Wv, bv, Wout, bout, dropout_mask):
    from concourse.bass_utils import run_bass_kernel_spmd

    x = np.ascontiguousarray(x, np.float32)
    m = np.asarray(attn_mask, np.float32).reshape(T, T)

    # causality check on the actual mask tensor
    causal = bool((np.tril(m) == 0).all() and
                  (m[np.triu_indices(T, 1)] <= -1e8).all())

    # safety: cheap bound on max |scaled score| -> exp overflow guard
    xf = x.reshape(B * T, D)
    Qa = xf @ Wq.transpose(1, 0, 2).reshape(D, H * DK)
    Ka = xf @ Wk.transpose(1, 0, 2).reshape(D, H * DK)
    Qa = Qa.reshape(B * T, H, DK) + bq[None]
    Ka = Ka.reshape(B * T, H, DK) + bk[None]
    qn = np.linalg.norm(Qa, axis=2).max(0)     # per-head max row norm
    kn = np.linalg.norm(Ka, axis=2).max(0)
    bound = float(SCALE) * float((qn * kn).max())
    if bound > 50.0:
        return _host_fallback(x, attn_mask, Wq, bq, Wk, bk, Wv, bv, Wout,
                              bout, dropout_mask)

    nc = _get_program(causal)

    xT = np.ascontiguousarray(x.transpose(2, 0, 1).reshape(D, B * T))
    woutT = np.ascontiguousarray(np.asarray(Wout, np.float32).T)
    boutr = np.asarray(bout, np.float32).reshape(1, D)
    idm = np.eye(128, dtype=np.float32)
    dmask = np.where(np.arange(128)[None, :] < np.arange(128)[:, None],
                     MASK_NEG, np.float32(0.0)).astype(np.float32)
    maskT = None if causal else np.ascontiguousarray(m.T * np.float32(8.0))
    drop = np.asarray(dropout_mask, np.float32)

    in_maps = []
    for c in range(NCORES):
        h0, h1 = HP * c, HP * c + 1
        im = {
            "xT": xT,
            "wq": np.ascontiguousarray(
                np.concatenate([Wq[h0], Wq[h1]], axis=1), np.float32),
            "wk": np.ascontiguousarray(
                np.concatenate([Wk[h0], Wk[h1]], axis=1), np.float32),
            "wv": np.ascontiguousarray(
                np.concatenate([Wv[h0], Wv[h1]], axis=1), np.float32),
            "bq": np.concatenate([bq[h0], bq[h1]]).reshape(128, 1)
                    .astype(np.float32),
            "bk": np.concatenate([bk[h0], bk[h1]]).reshape(128, 1)
                    .astype(np.float32),
            "bv": np.concatenate([bv[h0], bv[h1]]).reshape(128, 1)
                    .astype(np.float32),
            "wout": woutT,
            "bout": boutr,
            "drop": np.ascontiguousarray(
                drop[:, c * ROWS:(c + 1) * ROWS, :]),
            "idm": idm,
            "onesr": np.ones((1, 128), np.float32),
            "vcol": np.ones((128, 32), np.float32),
            "zer": np.zeros((128, 384), np.float32),
        }
        if causal:
            im["dmask"] = dmask
        else:
            im["maskT"] = maskT
        in_maps.append(im)

    res = run_bass_kernel_spmd(nc, in_maps, list(range(NCORES)))
    out = np.empty((B, T, D), np.float32)
    for c in range(NCORES):
        out[:, c * ROWS:(c + 1) * ROWS, :] = res.results[c]["out"]
    return out
